# revision 16
# baseline (speedup 1.0000x reference)
"""Sparse attention (per-query top-K) Trainium2 kernel, 8-core tensor-parallel.

v3 strategy (heads sharded 2-per-core, dense-score formulation):
  - Host folds idx/valid/geo_bias into per-(s,q) merged bias factors
    E[s,q] = sum_{j: idx[q,j]==s} exp(geo_bias[h,q,j]), stored as causal
    fp8(e3m4) tiles (scaled by 1/ESCALE; scale cancels in softmax).
        A^T = E^T * exp(S^T - C),   S^T = K @ Q^T (feature-major, bf16)
        out^T = [V | 1]^T @ A^T     (row 64 = softmax denominator)
  - Per core: Q/K/V projections for its 2 heads (x transposed on load by
    the DMA xbar, pipelined per strip; projections in bf16 with fp32
    accumulation), dense causal S^T on PE (bf16), exp on ACT (both heads
    per instruction via a 2-bank PSUM tile), E-multiply on DVE (fp16),
    AV on PE (fp16).
  - Query tiles processed big-first (t=7..0).  After each tile an
    AllGather reshards that tile's (unnormalized) head outputs +
    denominators to ALL cores; each core normalizes and computes its own
    128-COLUMN slice of the o_proj output for that tile (o_proj sharded
    by output columns), overlapped one tile behind phase 2.  Host
    concatenates the column slices and transposes.
"""

import sys

sys.path.insert(0, "/opt/trn_rl_repo")

import numpy as np
import ml_dtypes

from concourse import bacc, mybir, tile
from concourse.bass_utils import run_bass_kernel_spmd
from concourse.masks import make_identity

F32 = mybir.dt.float32
F32R = mybir.dt.float32r
F16 = mybir.dt.float16
BF16 = mybir.dt.bfloat16

S = 4096
H = 1024
NH = 16
KSEL = 32
HD = 64
NC = 8
HPC = NH // NC  # 2 heads per core
QT = 512
NQT = S // QT
SC = 128
CSHIFT = 2.0
SLAB = 16  # s-chunks per E-tile DMA slab

TILE_LIST = [(t, c) for t in range(NQT) for c in range(4 * (t + 1))]
N_TILES = len(TILE_LIST)  # 144
TILE_IDX = {tc: n for n, tc in enumerate(TILE_LIST)}

TILE_ORDER = list(range(NQT))  # small tiles first: exposed tail is AG(7)+P3(7)


def _build_program(n_reps=1, n_cores_build=NC):
    nc = bacc.Bacc(
        "TRN2", target_bir_lowering=False, debug=False, num_devices=n_cores_build
    )

    x_in = nc.dram_tensor("x", [S, H], BF16, kind="ExternalInput").ap()
    wq_in = nc.dram_tensor("wq", [H, 128], BF16, kind="ExternalInput").ap()
    wk_in = nc.dram_tensor("wk", [H, 128], BF16, kind="ExternalInput").ap()
    wv_in = nc.dram_tensor("wv", [H, 128], BF16, kind="ExternalInput").ap()
    wo_in = nc.dram_tensor("wo_cols", [H, 128], BF16, kind="ExternalInput").ap()
    bo_in = nc.dram_tensor("bo_col", [128, 1], F32, kind="ExternalInput").ap()
    e_in = nc.dram_tensor(
        "e_pack", [N_TILES, SC, HPC, QT], F16, kind="ExternalInput"
    ).ap()
    sel_in = nc.dram_tensor("sel16", [NH, H], BF16, kind="ExternalInput").ap()
    y_out = nc.dram_tensor("y_colT", [128, S], F32, kind="ExternalOutput").ap()

    with tile.TileContext(nc) as tc:
        with (
            tc.tile_pool(name="const", bufs=1) as constp,
            tc.tile_pool(name="persist", bufs=1) as persist,
            tc.tile_pool(name="dram", bufs=1, space="DRAM") as dram,
        ):
            ident = constp.tile([128, 128], F32, tag="ident")
            make_identity(nc, ident[:])
            nbias = constp.tile([128, 1], F32, tag="nbias")
            nc.gpsimd.memset(nbias[:], -CSHIFT)

            wq_sb = constp.tile([128, 8, 128], BF16, tag="wq")
            wk_sb = constp.tile([128, 8, 128], BF16, tag="wk")
            wv_sb = constp.tile([128, 8, 128], BF16, tag="wv")
            nc.sync.dma_start(wq_sb[:], wq_in.rearrange("(c p) m -> p c m", p=128))
            nc.sync.dma_start(wk_sb[:], wk_in.rearrange("(c p) m -> p c m", p=128))
            nc.sync.dma_start(wv_sb[:], wv_in.rearrange("(c p) m -> p c m", p=128))
            wo_sb = constp.tile([128, 8, 128], BF16, tag="wo")
            nc.sync.dma_start(wo_sb[:], wo_in.rearrange("(c p) m -> p c m", p=128))
            bo_sb = constp.tile([128, 1], F32, tag="bo")
            nc.sync.dma_start(bo_sb[:], bo_in[:])
            sel_sb = constp.tile([NH, H], BF16, tag="sel")
            nc.sync.dma_start(sel_sb[:], sel_in[:])

            qT_sb = persist.tile([128, NQT, QT], BF16, tag="qT")
            kT_sb = persist.tile([128, NQT, QT], BF16, tag="kT")
            v_sb = [
                persist.tile([128, S // SC, HD + 1], F16, tag=f"v{h}", name=f"v{h}")
                for h in range(HPC)
            ]
            for h in range(HPC):
                nc.gpsimd.memset(v_sb[h][:], 1.0)

            ag_in = dram.tile([NQT, NC, HPC * (HD + 1), QT], F16)
            ag_out = dram.tile([NQT, NC, HPC * (HD + 1), QT], F16)

            for _rep in range(n_reps):
                # ------------- phase 1: projections (feature-major) ---------
                with (
                    tc.tile_pool(name="xT", bufs=1) as xTp,
                    tc.tile_pool(name="vtmp", bufs=2) as vtmpp,
                    tc.tile_pool(name="p1ps", bufs=2, space="PSUM") as p1ps,
                    tc.tile_pool(name="p1projps", bufs=2, space="PSUM") as p1pp,
                ):
                    # one SBUF tile PER transposed strip: single writer each,
                    # so the framework doesn't serialize the transpose DMAs,
                    # and they spread across many DMA engines concurrently
                    xT_strip = [
                        [
                            xTp.tile(
                                [128, QT], BF16, tag=f"x{st}_{hc}",
                                name=f"x{st}_{hc}",
                            )
                            for hc in range(8)
                        ]
                        for st in range(NQT)
                    ]
                    for st in range(NQT):
                        for hc in range(8):
                            q = nc.sync if (st * 8 + hc) % 2 == 0 else nc.scalar
                            q.dma_start_transpose(
                                xT_strip[st][hc][:],
                                x_in[
                                    st * QT : (st + 1) * QT,
                                    hc * 128 : (hc + 1) * 128,
                                ],
                            )
                    for st in range(NQT):
                        ps_q = p1pp.tile([128, QT], F32, tag="psq")
                        ps_k = p1pp.tile([128, QT], F32, tag="psk")
                        ps_v = p1pp.tile([128, QT], F32, tag="psv")
                        for c in range(8):
                            nc.tensor.matmul(
                                ps_q[:], wq_sb[:, c, :], xT_strip[st][c][:],
                                start=(c == 0), stop=(c == 7),
                            )
                            nc.tensor.matmul(
                                ps_k[:], wk_sb[:, c, :], xT_strip[st][c][:],
                                start=(c == 0), stop=(c == 7),
                            )
                            nc.tensor.matmul(
                                ps_v[:], wv_sb[:, c, :], xT_strip[st][c][:],
                                start=(c == 0), stop=(c == 7),
                            )
                        nc.vector.tensor_copy(qT_sb[:, st, :], ps_q[:])
                        nc.vector.tensor_copy(kT_sb[:, st, :], ps_k[:])
                        vT_tmp = vtmpp.tile([128, QT], F32, tag="vt")
                        nc.scalar.copy(vT_tmp[:], ps_v[:])
                        ps_tv = p1ps.tile([128, QT], F32, tag="tp")
                        for i in range(4):
                            nc.tensor.transpose(
                                ps_tv[:, i * 128 : (i + 1) * 128],
                                vT_tmp[:, i * 128 : (i + 1) * 128],
                                ident[:],
                            )
                        ps_tv4 = ps_tv[:].rearrange("p (i h d) -> p i h d", i=4, h=HPC)
                        for h in range(HPC):
                            nc.vector.tensor_copy(
                                v_sb[h][:, st * 4 : (st + 1) * 4, 0:HD],
                                ps_tv4[:, :, h, :],
                            )

                # ------------- phase 2+3 interleaved ------------------------
                with (
                    tc.tile_pool(name="zap", bufs=6) as zap,
                    tc.tile_pool(name="ep", bufs=2) as epool,
                    tc.tile_pool(name="otp", bufs=2) as otp,
                    tc.tile_pool(name="p3sb", bufs=2) as p3sb,
                    tc.tile_pool(name="p3y", bufs=2) as p3y,
                    tc.tile_pool(name="p2s", bufs=2, space="PSUM") as p2s,
                    tc.tile_pool(name="p2o", bufs=1, space="PSUM") as p2o,
                    tc.tile_pool(name="p23", bufs=2, space="PSUM") as p23,
                ):
                    slab_no = 0

                    def phase2_tile(t):
                        nonlocal slab_no
                        nchunks = 4 * (t + 1)
                        slabs = []
                        for g0 in range(0, nchunks, SLAB):
                            gsz = min(SLAB, nchunks - g0)
                            e_slab = epool.tile(
                                [128, SLAB, HPC, QT], F16, tag="e", name="e_slab"
                            )
                            n0 = TILE_IDX[(t, g0)]
                            src = e_in[n0 : n0 + gsz].rearrange("n p h q -> p n h q")
                            # all on sync: the gpsimd queue blocks on collective
                            # triggers and the scalar queue must stay free for exp
                            nc.sync.dma_start(e_slab[:, 0:gsz, :, :], src)
                            slab_no += 1
                            slabs.append(e_slab)
                        ps_o = [
                            p2o.tile([HD + 1, QT], F32, tag=f"po{h}", name=f"po{h}")
                            for h in range(HPC)
                        ]
                        for c in range(nchunks):
                            e_slab = slabs[c // SLAB]
                            c_loc = c % SLAB
                            ps_s2 = p2s.tile([128, 2 * QT], F32, tag="ps2")
                            for h in range(HPC):
                                nc.tensor.matmul(
                                    ps_s2[:, h * QT : (h + 1) * QT],
                                    kT_sb[
                                        h * HD : (h + 1) * HD,
                                        c // 4,
                                        (c % 4) * 128 : (c % 4 + 1) * 128,
                                    ],
                                    qT_sb[h * HD : (h + 1) * HD, t, :],
                                    start=True,
                                    stop=True,
                                )
                            z_sb = zap.tile([128, HPC, QT], F16, tag="z")
                            nc.scalar.activation(
                                z_sb[:].rearrange("p h q -> p (h q)"),
                                ps_s2[:],
                                mybir.ActivationFunctionType.Exp,
                                bias=nbias[:],
                            )
                            a_sb = zap.tile([128, HPC, QT], F16, tag="a")
                            nc.vector.tensor_mul(
                                a_sb[:], z_sb[:], e_slab[:, c_loc, :, :]
                            )
                            for h in range(HPC):
                                nc.tensor.matmul(
                                    ps_o[h][:],
                                    v_sb[h][:, c, :],
                                    a_sb[:, h, :],
                                    start=(c == 0),
                                    stop=(c == nchunks - 1),
                                )
                        ot_sb = otp.tile([HD + 1, HPC, QT], F16, tag="ot")
                        for h in range(HPC):
                            nc.vector.tensor_copy(ot_sb[:, h, :], ps_o[h][:])
                        # emulate AllGather with a single-step AllToAll on a
                        # replicated input (the ring AllGather costs ~3x); the
                        # replication writes ride the gpsimd queue, which only
                        # hosts collective triggers anyway
                        for cdst in range(NC):
                            nc.gpsimd.dma_start(
                                ag_in[t][cdst].rearrange("(h p) q -> p h q", h=HPC),
                                ot_sb[:],
                            )
                        nc.gpsimd.collective_compute(
                            "AllToAll",
                            mybir.AluOpType.bypass,
                            replica_groups=[list(range(NC))],
                            ins=[ag_in[t].opt()],
                            outs=[ag_out[t].opt()],
                        )

                    def phase3_load(t):
                        # prefetch tile t's gathered heads while phase 2 of the
                        # next tile computes
                        den_sb = p3sb.tile([NH, QT], F16, tag="den")
                        oT_sb = p3sb.tile([128, 8, QT], F16, tag="oT")
                        for l in range(HPC):
                            nc.sync.dma_start(
                                den_sb[l * 8 : (l + 1) * 8, :],
                                ag_out[t][:, l * (HD + 1) + HD, :],
                            )
                            nc.sync.dma_start(
                                oT_sb[l * HD : (l + 1) * HD, :, :],
                                ag_out[t][
                                    :, l * (HD + 1) : l * (HD + 1) + HD, :
                                ].rearrange("c d q -> d c q"),
                            )
                        return den_sb, oT_sb

                    def phase3_compute(t, den_sb, oT_sb):
                        # normalize + my 128-column slice of o_proj for tile t
                        rden_sb = p3sb.tile([NH, QT], BF16, tag="rden")
                        with nc.allow_low_precision(reason="feeds bf16 matmul"):
                            nc.vector.reciprocal(rden_sb[:], den_sb[:])

                        on_sb = p3sb.tile([128, 8, QT], BF16, tag="on")
                        for ci in range(8):
                            ps_b = p23.tile([128, QT], F32, tag="p3ps")
                            nc.tensor.matmul(
                                ps_b[:],
                                sel_sb[:, ci * 128 : (ci + 1) * 128],
                                rden_sb[:],
                                start=True,
                                stop=True,
                            )
                            nc.vector.tensor_mul(
                                on_sb[:, ci, :], oT_sb[:, ci, :], ps_b[:]
                            )

                        ps_y = p23.tile([128, QT], F32, tag="p3ps")
                        for c in range(8):
                            nc.tensor.matmul(
                                ps_y[:],
                                wo_sb[:, c, :],
                                on_sb[:, c, :],
                                start=(c == 0),
                                stop=(c == 7),
                            )
                        yT_sb = p3y.tile([128, QT], F32, tag="y")
                        nc.vector.tensor_scalar(
                            yT_sb[:], ps_y[:], bo_sb[:], None, mybir.AluOpType.add
                        )
                        nc.sync.dma_start(y_out[:, t * QT : (t + 1) * QT], yT_sb[:])

                    pend = None
                    for t in TILE_ORDER:
                        phase2_tile(t)
                        if pend is not None:
                            # pend's AllGather finished a full tile ago -> the
                            # load DMAs don't block the sync queue
                            phase3_compute(pend, *phase3_load(pend))
                        pend = t
                    phase3_compute(pend, *phase3_load(pend))

    nc.compile()
    return nc


_PROGRAM_CACHE = {}


def _get_program():
    if "nc" not in _PROGRAM_CACHE:
        _PROGRAM_CACHE["nc"] = _build_program()
    return _PROGRAM_CACHE["nc"]


def _host_prep(x, idx, valid, geo_bias, Wq, Wk, Wv, Wo, bo):
    x2 = np.ascontiguousarray(np.asarray(x, dtype=np.float32).reshape(S, H))
    idx = np.asarray(idx).astype(np.int64)
    valid = np.asarray(valid).astype(bool)
    geo = np.asarray(geo_bias, dtype=np.float32)
    Wq = np.asarray(Wq, dtype=np.float32)
    Wk = np.asarray(Wk, dtype=np.float32)
    Wv = np.asarray(Wv, dtype=np.float32)
    Wo = np.asarray(Wo, dtype=np.float32)
    bo = np.asarray(bo, dtype=np.float32)

    qpos = np.arange(S, dtype=np.int64)[:, None]
    keep = valid & (idx <= qpos) & (idx >= 0)
    s_flat = idx[keep]
    q_flat = np.broadcast_to(qpos, idx.shape)[keep]
    lin = s_flat * S + q_flat

    # den row order in phase 3 is r = l*8 + ci for head h = 2*ci + l
    sel16 = np.zeros((NH, H), dtype=np.float32)
    ch = np.arange(H)
    sel16[((ch // HD) % 2) * 8 + ch // 128, ch] = 1.0
    sel16 = sel16.astype(ml_dtypes.bfloat16)

    wq_scaled = Wq / np.sqrt(HD)
    x_bf = x2.astype(ml_dtypes.bfloat16)
    wo_bf = Wo.astype(ml_dtypes.bfloat16)

    in_maps = []
    for core in range(NC):
        e_pack = np.empty((N_TILES, SC, HPC, QT), dtype=np.float16)
        for l in range(HPC):
            h = HPC * core + l
            w = np.exp(geo[h][keep].astype(np.float64))
            eT = np.bincount(lin, weights=w, minlength=S * S).reshape(S, S)
            for n, (t, c) in enumerate(TILE_LIST):
                e_pack[n, :, l, :] = eT[
                    c * SC : (c + 1) * SC, t * QT : (t + 1) * QT
                ].astype(np.float16)
        cs = slice(128 * core, 128 * (core + 1))
        in_maps.append(
            {
                "x": x_bf,
                "wq": np.ascontiguousarray(wq_scaled[:, cs]).astype(ml_dtypes.bfloat16),
                "wk": np.ascontiguousarray(Wk[:, cs]).astype(ml_dtypes.bfloat16),
                "wv": np.ascontiguousarray(Wv[:, cs]).astype(ml_dtypes.bfloat16),
                "wo_cols": np.ascontiguousarray(wo_bf[:, cs]),
                "bo_col": np.ascontiguousarray(bo[cs]).reshape(128, 1),
                "e_pack": e_pack,
                "sel16": sel16,
            }
        )
    return in_maps


LAST_RESULT = None


def kernel(x, idx, valid, geo_bias, Wq, Wk, Wv, Wo, bo):
    global LAST_RESULT
    b, s, h = np.asarray(x).shape
    assert (b, s, h) == (1, S, H)
    in_maps = _host_prep(x, idx, valid, geo_bias, Wq, Wk, Wv, Wo, bo)
    nc = _get_program()
    import os

    kwargs = {}
    if os.environ.get("KTRACE_DIR"):
        kwargs = dict(trace=True, tmpdir=os.environ["KTRACE_DIR"])
    res = run_bass_kernel_spmd(nc, in_maps, core_ids=list(range(NC)), **kwargs)
    LAST_RESULT = res
    yT = np.concatenate([res.results[c]["y_colT"] for c in range(NC)], axis=0)
    return np.ascontiguousarray(yT.T).reshape(1, S, H).astype(np.float32)


# revision 20
# speedup vs baseline: 1.2724x; 1.2724x over previous
"""Sparse attention (per-query top-K) Trainium2 kernel, 8-core tensor-parallel.

v3 strategy (heads sharded 2-per-core, dense-score formulation):
  - Host folds idx/valid/geo_bias into per-(s,q) merged bias factors
    E[s,q] = sum_{j: idx[q,j]==s} exp(geo_bias[h,q,j]), stored as causal
    fp8(e3m4) tiles (scaled by 1/ESCALE; scale cancels in softmax).
        A^T = E^T * exp(S^T - C),   S^T = K @ Q^T (feature-major, bf16)
        out^T = [V | 1]^T @ A^T     (row 64 = softmax denominator)
  - Per core: Q/K/V projections for its 2 heads (x transposed on load by
    the DMA xbar, pipelined per strip; projections in bf16 with fp32
    accumulation), dense causal S^T on PE (bf16), exp on ACT (both heads
    per instruction via a 2-bank PSUM tile), E-multiply on DVE (fp16),
    AV on PE (fp16).
  - Query tiles processed big-first (t=7..0).  After each tile an
    AllGather reshards that tile's (unnormalized) head outputs +
    denominators to ALL cores; each core normalizes and computes its own
    128-COLUMN slice of the o_proj output for that tile (o_proj sharded
    by output columns), overlapped one tile behind phase 2.  Host
    concatenates the column slices and transposes.
"""

import sys

sys.path.insert(0, "/opt/trn_rl_repo")

import numpy as np
import ml_dtypes

from concourse import bacc, mybir, tile
from concourse.bass_utils import run_bass_kernel_spmd
from concourse.masks import make_identity

F32 = mybir.dt.float32
F32R = mybir.dt.float32r
F16 = mybir.dt.float16
BF16 = mybir.dt.bfloat16

S = 4096
H = 1024
NH = 16
KSEL = 32
HD = 64
NC = 8
HPC = NH // NC  # 2 heads per core
QT = 512
NQT = S // QT
SC = 128
CSHIFT = 2.0
SLAB = 16  # s-chunks per E-tile DMA slab

TILE_LIST = [(t, c) for t in range(NQT) for c in range(4 * (t + 1))]
N_TILES = len(TILE_LIST)  # 144
TILE_IDX = {tc: n for n, tc in enumerate(TILE_LIST)}

TILE_ORDER = list(range(NQT))  # small tiles first: exposed tail is AG(7)+P3(7)


def _build_program(n_reps=1, n_cores_build=NC):
    nc = bacc.Bacc(
        "TRN2", target_bir_lowering=False, debug=False, num_devices=n_cores_build
    )

    x_in = nc.dram_tensor("x", [S, H], BF16, kind="ExternalInput").ap()
    wq_in = nc.dram_tensor("wq", [H, 128], BF16, kind="ExternalInput").ap()
    wk_in = nc.dram_tensor("wk", [H, 128], BF16, kind="ExternalInput").ap()
    wv_in = nc.dram_tensor("wv", [H, 128], BF16, kind="ExternalInput").ap()
    wo_in = nc.dram_tensor("wo_rows", [128, H], BF16, kind="ExternalInput").ap()
    e_in = nc.dram_tensor(
        "e_pack", [N_TILES, SC, HPC, QT], F16, kind="ExternalInput"
    ).ap()
    y_out = nc.dram_tensor("y_rs", [NQT, QT // NC, H], F16, kind="ExternalOutput").ap()

    with tile.TileContext(nc) as tc:
        with (
            tc.tile_pool(name="const", bufs=1) as constp,
            tc.tile_pool(name="persist", bufs=1) as persist,
            tc.tile_pool(name="dram", bufs=1, space="DRAM") as dram,
        ):
            ident = constp.tile([128, 128], F32, tag="ident")
            make_identity(nc, ident[:])
            nbias = constp.tile([128, 1], F32, tag="nbias")
            nc.gpsimd.memset(nbias[:], -CSHIFT)

            wq_sb = constp.tile([128, 8, 128], BF16, tag="wq")
            wk_sb = constp.tile([128, 8, 128], BF16, tag="wk")
            wv_sb = constp.tile([128, 8, 128], BF16, tag="wv")
            nc.sync.dma_start(wq_sb[:], wq_in.rearrange("(c p) m -> p c m", p=128))
            nc.sync.dma_start(wk_sb[:], wk_in.rearrange("(c p) m -> p c m", p=128))
            nc.sync.dma_start(wv_sb[:], wv_in.rearrange("(c p) m -> p c m", p=128))
            wo_sb = constp.tile([128, H], BF16, tag="wo")
            nc.sync.dma_start(wo_sb[:], wo_in[:])
            ones1 = constp.tile([1, 128], BF16, tag="ones1")
            nc.gpsimd.memset(ones1[:], 1.0)

            qT_sb = persist.tile([128, NQT, QT], BF16, tag="qT")
            kT_sb = persist.tile([128, NQT, QT], BF16, tag="kT")
            v_sb = [
                persist.tile([128, S // SC, HD + 1], F16, tag=f"v{h}", name=f"v{h}")
                for h in range(HPC)
            ]
            for h in range(HPC):
                nc.gpsimd.memset(v_sb[h][:], 1.0)

            o_loc = persist.tile([128, NQT, QT], F16, tag="oloc")
            # per-head denominator tiles, each at partition 0 (the BIR
            # verifier rejects partition starts other than 0/32/64/96)
            den_loc = [
        persist.tile([1, NQT, QT], F16, tag=f"den{h}", name=f"den{h}")
                for h in range(HPC)
            ]
            yp_dram = dram.tile([NQT, QT, H], F16)
            y_rs_buf = dram.tile([NQT, QT // NC, H], F16)

            for _rep in range(n_reps):
                # ------------- phase 1: projections (feature-major) ---------
                with (
                    tc.tile_pool(name="xT", bufs=1) as xTp,
                    tc.tile_pool(name="vtmp", bufs=2) as vtmpp,
                    tc.tile_pool(name="p1ps", bufs=2, space="PSUM") as p1ps,
                    tc.tile_pool(name="p1projps", bufs=2, space="PSUM") as p1pp,
                ):
                    # one SBUF tile PER transposed strip: single writer each,
                    # so the framework doesn't serialize the transpose DMAs,
                    # and they spread across many DMA engines concurrently
                    xT_strip = [
                        [
                            xTp.tile(
                                [128, QT], BF16, tag=f"x{st}_{hc}",
                                name=f"x{st}_{hc}",
                            )
                            for hc in range(8)
                        ]
                        for st in range(NQT)
                    ]
                    for st in range(NQT):
                        for hc in range(8):
                            q = nc.sync if (st * 8 + hc) % 2 == 0 else nc.scalar
                            q.dma_start_transpose(
                                xT_strip[st][hc][:],
                                x_in[
                                    st * QT : (st + 1) * QT,
                                    hc * 128 : (hc + 1) * 128,
                                ],
                            )
                    for st in range(NQT):
                        ps_q = p1pp.tile([128, QT], F32, tag="psq")
                        ps_k = p1pp.tile([128, QT], F32, tag="psk")
                        ps_v = p1pp.tile([128, QT], F32, tag="psv")
                        for c in range(8):
                            nc.tensor.matmul(
                                ps_q[:], wq_sb[:, c, :], xT_strip[st][c][:],
                                start=(c == 0), stop=(c == 7),
                            )
                            nc.tensor.matmul(
                                ps_k[:], wk_sb[:, c, :], xT_strip[st][c][:],
                                start=(c == 0), stop=(c == 7),
                            )
                            nc.tensor.matmul(
                                ps_v[:], wv_sb[:, c, :], xT_strip[st][c][:],
                                start=(c == 0), stop=(c == 7),
                            )
                        nc.vector.tensor_copy(qT_sb[:, st, :], ps_q[:])
                        nc.vector.tensor_copy(kT_sb[:, st, :], ps_k[:])
                        vT_tmp = vtmpp.tile([128, QT], F32, tag="vt")
                        nc.scalar.copy(vT_tmp[:], ps_v[:])
                        ps_tv = p1ps.tile([128, QT], F32, tag="tp")
                        for i in range(4):
                            nc.tensor.transpose(
                                ps_tv[:, i * 128 : (i + 1) * 128],
                                vT_tmp[:, i * 128 : (i + 1) * 128],
                                ident[:],
                            )
                        ps_tv4 = ps_tv[:].rearrange("p (i h d) -> p i h d", i=4, h=HPC)
                        for h in range(HPC):
                            nc.vector.tensor_copy(
                                v_sb[h][:, st * 4 : (st + 1) * 4, 0:HD],
                                ps_tv4[:, :, h, :],
                            )

                # ------------- phase 2+3 interleaved ------------------------
                with (
                    tc.tile_pool(name="zap", bufs=6) as zap,
                    tc.tile_pool(name="ep", bufs=2) as epool,
                    tc.tile_pool(name="otp", bufs=2) as otp,
                    tc.tile_pool(name="p3sb", bufs=2) as p3sb,
                    tc.tile_pool(name="p3y", bufs=2) as p3y,
                    tc.tile_pool(name="p2s", bufs=2, space="PSUM") as p2s,
                    tc.tile_pool(name="p2o", bufs=1, space="PSUM") as p2o,
                    tc.tile_pool(name="p23", bufs=2, space="PSUM") as p23,
                ):
                    slab_no = 0

                    def phase2_tile(t):
                        nonlocal slab_no
                        nchunks = 4 * (t + 1)
                        slabs = []
                        for g0 in range(0, nchunks, SLAB):
                            gsz = min(SLAB, nchunks - g0)
                            e_slab = epool.tile(
                                [128, SLAB, HPC, QT], F16, tag="e", name="e_slab"
                            )
                            n0 = TILE_IDX[(t, g0)]
                            src = e_in[n0 : n0 + gsz].rearrange("n p h q -> p n h q")
                            # all on sync: the gpsimd queue blocks on collective
                            # triggers and the scalar queue must stay free for exp
                            nc.sync.dma_start(e_slab[:, 0:gsz, :, :], src)
                            slab_no += 1
                            slabs.append(e_slab)
                        ps_o = [
                            p2o.tile([HD + 1, QT], F32, tag=f"po{h}", name=f"po{h}")
                            for h in range(HPC)
                        ]
                        for c in range(nchunks):
                            e_slab = slabs[c // SLAB]
                            c_loc = c % SLAB
                            # diagonal chunks: keys >= some of the tile's
                            # queries, so only queries [qlo:] can attend
                            import os as _os
                            if _os.environ.get('KPROBE_VARIANT') == 'noqlo':
                                qlo = 0
                            else:
                                qlo = max(0, 128 * c - QT * t)
                            ps_s2 = p2s.tile([128, 2 * QT], F32, tag="ps2")
                            ps_s2v = ps_s2[:].rearrange("p (h q) -> p h q", h=HPC)
                            for h in range(HPC):
                                nc.tensor.matmul(
                                    ps_s2[:, h * QT + qlo : (h + 1) * QT],
                                    kT_sb[
                                        h * HD : (h + 1) * HD,
                                        c // 4,
                                        (c % 4) * 128 : (c % 4 + 1) * 128,
                                    ],
                                    qT_sb[h * HD : (h + 1) * HD, t, qlo:],
                                    start=True,
                                    stop=True,
                                )
                            z_sb = zap.tile([128, HPC, QT], F16, tag="z")
                            nc.scalar.activation(
                                z_sb[:, :, qlo:],
                                ps_s2v[:, :, qlo:],
                                mybir.ActivationFunctionType.Exp,
                                bias=nbias[:],
                            )
                            a_sb = zap.tile([128, HPC, QT], F16, tag="a")
                            nc.vector.tensor_mul(
                                a_sb[:, :, qlo:],
                                z_sb[:, :, qlo:],
                                e_slab[:, c_loc, :, qlo:],
                            )
                            for h in range(HPC):
                                nc.tensor.matmul(
                                    ps_o[h][:, qlo:],
                                    v_sb[h][:, c, :],
                                    a_sb[:, h, qlo:],
                                    start=(c == 0),
                                    stop=(c == nchunks - 1),
                                )
                        for h in range(HPC):
                            nc.vector.tensor_copy(
                                o_loc[h * HD : (h + 1) * HD, t, :],
                                ps_o[h][0:HD, :],
                            )
                            nc.vector.tensor_copy(
                                den_loc[h][0:1, t, :], ps_o[h][HD : HD + 1, :]
                            )

                    def phase3_tile(t):
                        # local per-head normalization (denominators are
                        # per-head, so no cross-core data is needed), then my
                        # 2 heads' partial contribution to o_proj for ALL H
                        # columns; ReduceScatter sums partials across cores
                        # with no on-device consumer -> PE never waits on CC
                        on_sb = p3sb.tile([128, QT], BF16, tag="on")
                        for h in range(HPC):
                            rden_sb = p3sb.tile([1, QT], BF16, tag=f"rd{h}")
                            with nc.allow_low_precision(reason="bf16 matmul"):
                                nc.vector.reciprocal(
                                    rden_sb[:], den_loc[h][:, t, :]
                                )
                            # outer-product broadcast of rden over partitions
                            ps_b = p23.tile([128, QT], F32, tag="p3ps")
                            nc.tensor.matmul(
                                ps_b[:], ones1[:], rden_sb[:],
                                start=True, stop=True,
                            )
                            nc.vector.tensor_mul(
                                on_sb[h * HD : (h + 1) * HD, :],
                                o_loc[h * HD : (h + 1) * HD, t, :],
                                ps_b[h * HD : (h + 1) * HD, :],
                            )
                        yp_sb = p3y.tile([128, 4, 2, QT], F16, tag="yp")
                        for qc in range(4):
                            for hh in range(2):
                                ps_y = p23.tile([128, QT], F32, tag="p3ps")
                                nc.tensor.matmul(
                                    ps_y[:],
                                    on_sb[:, qc * 128 : (qc + 1) * 128],
                                    wo_sb[:, hh * QT : (hh + 1) * QT],
                                    start=True,
                                    stop=True,
                                )
                                nc.vector.tensor_copy(yp_sb[:, qc, hh, :], ps_y[:])
                        nc.sync.dma_start(
                            yp_dram[t].rearrange(
                                "(qc p) (hh m) -> p qc hh m", p=128, hh=2
                            ),
                            yp_sb[:],
                        )
                        import os as _os
                        if _os.environ.get('KPROBE_VARIANT') != 'nors':
                            nc.gpsimd.collective_compute(
                                "ReduceScatter",
                                mybir.AluOpType.add,
                                replica_groups=[list(range(NC))],
                                ins=[yp_dram[t].opt()],
                                outs=[y_rs_buf[t].opt()],
                            )

                    for i, t in enumerate(TILE_ORDER):
                        phase2_tile(t)
                        phase3_tile(t)
                        if i >= 2:
                            # drain an RS result that completed tiles ago (the
                            # compiler rejects RS directly into ExternalOutput)
                            td = TILE_ORDER[i - 2]
                            nc.sync.dma_start(y_out[td], y_rs_buf[td])
                    for t in TILE_ORDER[-2:]:
                        nc.sync.dma_start(y_out[t], y_rs_buf[t])

    nc.compile()
    return nc


_PROGRAM_CACHE = {}


def _get_program():
    if "nc" not in _PROGRAM_CACHE:
        _PROGRAM_CACHE["nc"] = _build_program()
    return _PROGRAM_CACHE["nc"]


def _host_prep(x, idx, valid, geo_bias, Wq, Wk, Wv, Wo, bo):
    x2 = np.ascontiguousarray(np.asarray(x, dtype=np.float32).reshape(S, H))
    idx = np.asarray(idx).astype(np.int64)
    valid = np.asarray(valid).astype(bool)
    geo = np.asarray(geo_bias, dtype=np.float32)
    Wq = np.asarray(Wq, dtype=np.float32)
    Wk = np.asarray(Wk, dtype=np.float32)
    Wv = np.asarray(Wv, dtype=np.float32)
    Wo = np.asarray(Wo, dtype=np.float32)
    bo = np.asarray(bo, dtype=np.float32)

    qpos = np.arange(S, dtype=np.int64)[:, None]
    keep = valid & (idx <= qpos) & (idx >= 0)
    s_flat = idx[keep]
    q_flat = np.broadcast_to(qpos, idx.shape)[keep]
    lin = s_flat * S + q_flat

    wq_scaled = Wq / np.sqrt(HD)
    x_bf = x2.astype(ml_dtypes.bfloat16)
    wo_bf = Wo.astype(ml_dtypes.bfloat16)

    in_maps = []
    for core in range(NC):
        e_pack = np.empty((N_TILES, SC, HPC, QT), dtype=np.float16)
        for l in range(HPC):
            h = HPC * core + l
            w = np.exp(geo[h][keep].astype(np.float64))
            eT = np.bincount(lin, weights=w, minlength=S * S).reshape(S, S)
            for n, (t, c) in enumerate(TILE_LIST):
                e_pack[n, :, l, :] = eT[
                    c * SC : (c + 1) * SC, t * QT : (t + 1) * QT
                ].astype(np.float16)
        cs = slice(128 * core, 128 * (core + 1))
        in_maps.append(
            {
                "x": x_bf,
                "wq": np.ascontiguousarray(wq_scaled[:, cs]).astype(ml_dtypes.bfloat16),
                "wk": np.ascontiguousarray(Wk[:, cs]).astype(ml_dtypes.bfloat16),
                "wv": np.ascontiguousarray(Wv[:, cs]).astype(ml_dtypes.bfloat16),
                "wo_rows": np.ascontiguousarray(wo_bf[cs, :]),
                "e_pack": e_pack,
            }
        )
    return in_maps, bo


LAST_RESULT = None


def kernel(x, idx, valid, geo_bias, Wq, Wk, Wv, Wo, bo):
    global LAST_RESULT
    b, s, h = np.asarray(x).shape
    assert (b, s, h) == (1, S, H)
    in_maps, bo_np = _host_prep(x, idx, valid, geo_bias, Wq, Wk, Wv, Wo, bo)
    nc = _get_program()
    import os

    kwargs = {}
    if os.environ.get("KTRACE_DIR"):
        kwargs = dict(trace=True, tmpdir=os.environ["KTRACE_DIR"])
    res = run_bass_kernel_spmd(nc, in_maps, core_ids=list(range(NC)), **kwargs)
    LAST_RESULT = res
    # core c's y_rs[t] holds summed rows [t*QT + c*64 : t*QT + (c+1)*64]
    y = np.empty((NQT, NC, QT // NC, H), dtype=np.float32)
    for c in range(NC):
        y[:, c] = np.asarray(res.results[c]["y_rs"], dtype=np.float32)
    y = y.reshape(S, H) + bo_np[None, :]
    return y.reshape(1, S, H)
